# revision 7
# baseline (speedup 1.0000x reference)
"""Bass/Tile TRN2 kernel for nn_CropDrones — batched bbox + pair-gather tail.

Op: per-sample, find bbox of a rectangular binary window mask (channel 3 of
input1), crop rows [r0, r1) x cols [c0, c1) of the 3 image channels, paste the
crop centered into a 256x256 zero canvas.

Sharding: data parallel, 8 cores x 4 samples.

Device strategy:
  Phase 1 (all 4 samples batched; partition p = 32i + q is sample i, band q):
    ONE coarse DMA loads mask rows 16q -> mc [128, 512]. Row sums (ACT
    accum) + first moments (DVE accum). Rectangle mask => per-group SUMS
    recover everything. Stats are placed into per-sample column blocks
    (candw) and ONE PE matmul with an all-ones matrix broadcasts every
    sample's sums to every partition as columns: S3=band count, S4=sum q,
    S1=cnt*s, S2=cnt*m -> k1/k2 (boundary bands), s=sw+1, c0+c1=2*S2/S1.
  Refine: ONE indirect DMA reads one mask element at (row, cmid) for the 32
    boundary rows of each sample (cmid=(c0+c1)//2 is inside the rectangle).
    Counts via a second all-ones matmul give exact r0/r1 by sums only.
  Tail (partition p holds output rows 2p, 2p+1 of sample i per DMA):
    12 indirect gathers (sample x channel), ONE element-granular index per
    partition (coef patched to 1), each reading a contiguous 768-float run
    = rows (rt+2p, rt+2p+1) at cols d..: output pair rows at run offsets
    [0,256) and [512,768). Masks built per sample on ACT (scale trick);
    fin = gat*msk on DVE; 4 batched stores (one per sample, 3 channels).
"""

import numpy as np

import concourse.bass as bass
import concourse.bacc as bacc
import concourse.mybir as mybir
from concourse.bass import IndirectOffsetOnAxis
from concourse.bass_utils import run_bass_kernel_spmd
from concourse.tile import TileContext

B, C, H, W = 32, 3, 512, 512
CH_IN = 4
S = 256
N_CORES = 8
BPC = B // N_CORES   # 4 samples per core
P = 128
HW = H * W           # 262144
CHW4 = CH_IN * HW    # 1048576
NEL = BPC * CHW4     # 4194304
SS = S * S
GL = 2 * W - S       # 768: pair-run length

f32 = mybir.dt.float32
f16 = mybir.dt.float16
i32 = mybir.dt.int32
Alu = mybir.AluOpType
Act = mybir.ActivationFunctionType

_C_WIDTHS = {
    "c_q": 1,      # q = p % 32
    "c_qr": 1,     # q<16 ? q-15 : q-16   (refine row local offset)
    "c_lo16": 1,   # q < 16
    "c_hi16": 1,   # 16 <= q < 32
    "c_basem": 1,  # (p//32)*CHW4 + 3*HW - 0.25 (refine base, cmid round bias)
    "c_2p": 1,     # 2p
    "c_y2": 2,     # 2p + k
    "c_bm4": 4,    # (p//32 == col)
    "c_bmlo": 4,   # (p//32 == col) * (q < 16)
    "c_bmhi": 4,   # (p//32 == col) * (q >= 16)
    "c_base4": 4,  # col * CHW4
    "c_w32": P,    # block-diag ones: (p//32 == m//32)
    "c_iw": W,     # iota 0..511
    "c_w1": P,     # all-ones
}
_C_OFFS = {}
_off = 0
for _k, _w in _C_WIDTHS.items():
    _C_OFFS[_k] = _off
    _off += _w
C_TOTAL = _off


def _consts() -> dict[str, np.ndarray]:
    p = np.arange(P)
    q = p % 32
    g = p // 32
    vals = {
        "c_q": q[:, None].astype(np.float32),
        "c_qr": np.where(q < 16, q - 15, q - 16)[:, None].astype(np.float32),
        "c_lo16": (q < 16)[:, None].astype(np.float32),
        "c_hi16": (q >= 16)[:, None].astype(np.float32),
        "c_basem": (g * CHW4 + 3 * HW - 0.25)[:, None].astype(np.float32),
        "c_2p": (2.0 * p[:, None]).astype(np.float32),
        "c_y2": (2 * p[:, None] + np.arange(2)[None, :]).astype(np.float32),
        "c_bm4": (g[:, None] == np.arange(4)[None, :]).astype(np.float32),
        "c_bmlo": ((g[:, None] == np.arange(4)[None, :]) & (q < 16)[:, None]
                   ).astype(np.float32),
        "c_bmhi": ((g[:, None] == np.arange(4)[None, :]) & (q >= 16)[:, None]
                   ).astype(np.float32),
        "c_base4": np.broadcast_to(
            (np.arange(4) * CHW4).astype(np.float32), (P, 4)).copy(),
        "c_w32": (g[:, None] == (np.arange(P)[None, :] // 32)).astype(np.float32),
        "c_iw": np.broadcast_to(np.arange(W, dtype=np.float32), (P, W)).copy(),
        "c_w1": np.ones((P, P), dtype=np.float32),
    }
    packed = np.zeros((P, C_TOTAL), dtype=np.float32)
    for kk, vv in vals.items():
        packed[:, _C_OFFS[kk] : _C_OFFS[kk] + _C_WIDTHS[kk]] = vv
    return {"c_all": packed}


def _patch_coef1(binst):
    """Make the indirect DMA's indices element-granular (coef GL -> 1)."""
    a0 = binst.ins.ins[0]
    d0 = a0.dynamic_ap_info
    a0.dynamic_ap_info = mybir.DynamicAccessPatternInfo(
        c=d0.c, actual_ap=d0.actual_ap,
        indirect_dim_max_index=d0.indirect_dim_max_index,
        offset_expr=[mybir.DynamicAccessPatternOffsetExpr(
            coef=1, aff_expr=d0.offset_expr[0].aff_expr)])


def _colview(tile_ap, offset_col: int, stride: int, n: int) -> bass.AP:
    """[128, n] view of every `stride`-th column starting at offset_col."""
    v = tile_ap
    return bass.AP(v.tensor, v.offset + offset_col,
                   [list(v.ap[0]), [stride, n]])


def _build(debug: bool = False) -> bass.Bass:
    nc = bacc.Bacc("TRN2")
    x = nc.dram_tensor("x", [BPC, CH_IN, H, W], f32, kind="ExternalInput")
    y = nc.dram_tensor("y", [BPC, C, S, S], f32, kind="ExternalOutput")
    c_allB = nc.dram_tensor("c_allB", [P, C_TOTAL - W - P], f32,
                            kind="ExternalInput")
    if debug:
        dbg = nc.dram_tensor("dbg", [P, 80], f32, kind="ExternalOutput")

    ts = nc.vector.tensor_scalar
    st = nc.vector.scalar_tensor_tensor
    tt = nc.vector.tensor_tensor
    cpy = nc.vector.tensor_copy
    act = nc.scalar.activation
    tsp = nc.gpsimd.tensor_scalar
    ttp = nc.gpsimd.tensor_tensor
    cpyp = nc.gpsimd.tensor_copy

    with TileContext(nc) as tc:
        with (
            tc.tile_pool(name="consts", bufs=1) as cpool,
            tc.tile_pool(name="work", bufs=1) as wp,
            tc.tile_pool(name="psum", bufs=1, space="PSUM") as pp,
        ):
            # ---- phase 0: loads (small consts first, then coarse mask) ----
            mc = wp.tile([P, W], f16, tag="mc", name="mc")
            nc.gpsimd.dma_start(
                mc[:],
                bass.AP(x[:].tensor, 3 * HW,
                        [[CHW4, BPC], [16 * W, 32], [1, W]]),
            )
            ctB = cpool.tile([P, C_TOTAL - W - P], f32, tag="cB", name="cB")
            nc.sync.dma_start(ctB[:], c_allB[:])
            # generate iota 0..511 (f32) and the all-ones weights on-device
            ctA = cpool.tile([P, W], f32, tag="cA", name="cA")
            ctAi = cpool.tile([P, W], i32, tag="cAi", name="cAi")
            nc.gpsimd.iota(ctAi[:], [[1, W]], base=0, channel_multiplier=0)
            cpy(ctA[:], ctAi[:])
            ctC = cpool.tile([P, P], f32, tag="cC", name="cC")
            nc.vector.memset(ctC[:], 1.0)
            c_iw = ctA[:]
            c_w1 = ctC[:]

            def cb(key):
                off = _C_OFFS[key]
                return ctB[:, off : off + _C_WIDTHS[key]]

            c_q, c_qr = cb("c_q"), cb("c_qr")
            c_lo16, c_hi16 = cb("c_lo16"), cb("c_hi16")
            c_basem, c_2p, c_y2 = cb("c_basem"), cb("c_2p"), cb("c_y2")
            c_bm4, c_base4 = cb("c_bm4"), cb("c_base4")
            c_bmlo, c_bmhi = cb("c_bmlo"), cb("c_bmhi")
            c_w32 = cb("c_w32")

            # warm ACT function table immediately
            warmt = wp.tile([P, 1], f32, tag="warmt", name="warmt")
            nc.vector.memset(warmt[:], 0.0)
            act(warmt[:], warmt[:], Act.Identity, scale=1.0)

            # ---- phase 1: coarse stats ------------------------------------
            # cand cols: [bandany, srow, q*any, m~] = per-band [S3c,S1c,S4c,S2c]
            cand = wp.tile([P, 4], f32, tag="cand", name="cand")
            junka = wp.tile([P, W], f32, tag="junka", name="junka")
            junkb = wp.tile([P, W], f32, tag="junkb", name="junkb")
            act(junka[:], mc[:], Act.Identity, scale=1.0,
                accum_out=cand[:, 1:2])
            st(junkb[:], mc[:], 1.0, c_iw,
               op0=Alu.mult, op1=Alu.mult, accum_out=cand[:, 3:4])
            # bandany from the moment (m~ > 0 iff band row intersects the
            # window, since sw >= 32) keeps the ACT srow off the chain
            ts(cand[:, 0:1], cand[:, 3:4], 0.0, None, op0=Alu.is_gt)
            tt(cand[:, 2:3], cand[:, 0:1], c_q, op=Alu.mult)

            # ---- refine-critical chain: group-LOCAL stats via block-diag mm
            hp1 = tc.high_priority(offset=2000)
            hp1.__enter__()
            psumL = pp.tile([P, 4], f32, tag="psL", name="psL")
            nc.tensor.matmul(psumL[:], c_w32, cand[:])
            stL = wp.tile([P, 4], f32, tag="stL", name="stL")
            cpy(stL[:], psumL[:])      # [S3, S1, S4, S2] per group
            rcpL = wp.tile([P, 2], f32, tag="rcpL", name="rcpL")
            nc.vector.reciprocal(rcpL[:], stL[:, 0:2])
            tL = wp.tile([P, 6], f32, tag="tL", name="tL")
            # tL cols: 0 t=S4/S3, 1 u=(1-S3)/2, 2 k1L, 3 k2L, 4 cmL, 5 selA
            tt(tL[:, 0:1], stL[:, 2:3], rcpL[:, 0:1], op=Alu.mult)
            ts(tL[:, 1:2], stL[:, 0:1], -0.5, 0.5, op0=Alu.mult, op1=Alu.add)
            tt(tL[:, 2:3], tL[:, 0:1], tL[:, 1:2], op=Alu.add)       # k1L
            tt(tL[:, 3:4], tL[:, 0:1], tL[:, 1:2], op=Alu.subtract)  # k2L
            tt(tL[:, 4:5], stL[:, 3:4], rcpL[:, 1:2], op=Alu.mult)   # cmL
            tt(tL[:, 5:6], tL[:, 2:3], c_lo16, op=Alu.mult)
            tt(tL[:, 0:1], tL[:, 3:4], c_hi16, op=Alu.mult)
            tt(tL[:, 5:6], tL[:, 5:6], tL[:, 0:1], op=Alu.add)       # sel
            rowt = wp.tile([P, 1], f32, tag="rowt", name="rowt")
            rfi = wp.tile([P, 1], i32, tag="rfi", name="rfi")
            ts(rowt[:], tL[:, 5:6], 16.0, c_qr, op0=Alu.mult, op1=Alu.add)
            ts(rowt[:], rowt[:], 0.0, None, op0=Alu.max)
            ts(rowt[:], rowt[:], 512.0, tL[:, 4:5], op0=Alu.mult, op1=Alu.add)
            tt(rowt[:], rowt[:], c_basem, op=Alu.add)   # basem has -0.25 bias
            cpy(rfi[:], rowt[:])
            hp1.__exit__(None, None, None)

            rf = wp.tile([P, 1], f32, tag="rf", name="rf")
            nc.gpsimd.indirect_dma_start(
                out=rf[:],
                out_offset=None,
                in_=bass.AP(x[:].tensor, 0, [[1, NEL], [1, 1]]),
                in_offset=IndirectOffsetOnAxis(ap=rfi[:], axis=0),
            )

            # ---- column path: all-sample broadcast (overlaps refine) ------
            candw = wp.tile([P, 16], f32, tag="candw", name="candw")
            for j in range(4):
                ts(_colview(candw[:], j, 4, 4), c_bm4, cand[:, j : j + 1],
                   None, op0=Alu.mult)
            psum1 = pp.tile([P, 16], f32, tag="ps1", name="ps1")
            nc.tensor.matmul(psum1[:], c_w1, candw[:])
            st1 = wp.tile([P, 16], f32, tag="st1", name="st1")
            cpy(st1[:], psum1[:])   # cols 4i+[S3,S1,S4,S2]
            S3v = _colview(st1[:], 0, 4, 4)
            S1v = _colview(st1[:], 1, 4, 4)
            S4v = _colview(st1[:], 2, 4, 4)
            S2v = _colview(st1[:], 3, 4, 4)
            rcpw = wp.tile([P, 8], f32, tag="rcpw", name="rcpw")
            nc.vector.reciprocal(
                rcpw[:],
                bass.AP(st1[:].tensor, st1[:].offset,
                        [list(st1[:].ap[0]), [4, 4], [1, 2]]))
            r3v = _colview(rcpw[:], 0, 2, 4)
            r1v = _colview(rcpw[:], 1, 2, 4)
            # scw: 0:4 ksum, 4:8 s, 8:12 csum (adjacent for batched round)
            scw = wp.tile([P, 12], f32, tag="scw", name="scw")
            sciw = wp.tile([P, 12], i32, tag="sciw", name="sciw")
            tt(scw[:, 0:4], S4v, r3v, op=Alu.mult)
            ts(scw[:, 0:4], scw[:, 0:4], 2.0, None, op0=Alu.mult)
            tt(scw[:, 4:8], S1v, r3v, op=Alu.mult)
            tt(scw[:, 8:12], S2v, r1v, op=Alu.mult)
            ts(scw[:, 8:12], scw[:, 8:12], 2.0, None, op0=Alu.mult)
            cpy(sciw[:], scw[:])
            cpy(scw[:], sciw[:])    # ksum4, s4, csum4 exact ints
            k1_4 = wp.tile([P, 4], f32, tag="k1", name="k1")
            k2_4 = wp.tile([P, 4], f32, tag="k2", name="k2")
            tt(k1_4[:], scw[:, 0:4], S3v, op=Alu.subtract)
            ts(k1_4[:], k1_4[:], 0.5, 0.5, op0=Alu.mult, op1=Alu.add)
            tt(k2_4[:], scw[:, 0:4], S3v, op=Alu.add)
            ts(k2_4[:], k2_4[:], 0.5, -0.5, op0=Alu.mult, op1=Alu.add)

            # ---- phase B: refine-independent work (overlaps refine) -------
            left4 = wp.tile([P, 4], f32, tag="left4", name="left4")
            lefti = wp.tile([P, 4], i32, tag="lefti", name="lefti")
            lw4 = wp.tile([P, 4], f32, tag="lw4", name="lw4")
            c04 = wp.tile([P, 4], f32, tag="c04", name="c04")
            db4 = wp.tile([P, 4], f32, tag="db4", name="db4")
            kp4 = wp.tile([P, 4], f32, tag="kp4", name="kp4")
            km4 = wp.tile([P, 4], f32, tag="km4", name="km4")
            ts(left4[:], scw[:, 4:8], -0.5, 128.25, op0=Alu.mult, op1=Alu.add)
            cpy(lefti[:], left4[:])
            cpy(left4[:], lefti[:])
            st(lw4[:], left4[:], -1.0, scw[:, 4:8],
               op0=Alu.add, op1=Alu.add)
            tt(c04[:], scw[:, 8:12], scw[:, 4:8], op=Alu.subtract)
            ts(c04[:], c04[:], 0.5, 0.5, op0=Alu.mult, op1=Alu.add)
            tt(db4[:], c04[:], left4[:], op=Alu.subtract)
            tt(db4[:], db4[:], c_base4, op=Alu.add)
            ts(kp4[:], k1_4[:], 16.0, 1.0, op0=Alu.mult, op1=Alu.add)
            ts(km4[:], k2_4[:], 16.0, -1.0, op0=Alu.mult, op1=Alu.add)


            # ---- phase C: exact bbox from refine counts -------------------
            hpc = tc.high_priority(offset=2000)
            hpc.__enter__()
            cand2w = wp.tile([P, 8], f32, tag="cand2w", name="cand2w")
            # cand2w[:, 2i+j] = rf * (bm4 & lo16|hi16), one op per parity
            ts(_colview(cand2w[:], 0, 2, 4), c_bmlo, rf[:], None,
               op0=Alu.mult)
            ts(_colview(cand2w[:], 1, 2, 4), c_bmhi, rf[:], None,
               op0=Alu.mult)
            psum2 = pp.tile([P, 8], f32, tag="ps2", name="ps2")
            nc.tensor.matmul(psum2[:], c_w1, cand2w[:])
            st2 = wp.tile([P, 8], f32, tag="st2", name="st2")
            cpy(st2[:], psum2[:])   # cols 2i+[c1cnt, c2cnt]

            r0_4 = wp.tile([P, 4], f32, tag="r0", name="r0")
            r1_4 = wp.tile([P, 4], f32, tag="r1", name="r1")
            sh4 = wp.tile([P, 4], f32, tag="sh4", name="sh4")
            top4 = wp.tile([P, 4], f32, tag="top4", name="top4")
            topi = wp.tile([P, 4], i32, tag="topi", name="topi")
            rt4 = wp.tile([P, 4], f32, tag="rt4", name="rt4")
            tph4 = wp.tile([P, 4], f32, tag="tph4", name="tph4")
            tt(r0_4[:], kp4[:], _colview(st2[:], 0, 2, 4), op=Alu.subtract)
            ts(r0_4[:], r0_4[:], 0.0, None, op0=Alu.max)
            tt(r1_4[:], km4[:], _colview(st2[:], 1, 2, 4), op=Alu.add)
            tt(sh4[:], r1_4[:], r0_4[:], op=Alu.subtract)
            ts(top4[:], sh4[:], -0.5, 127.75, op0=Alu.mult, op1=Alu.add)
            cpy(topi[:], top4[:])
            cpy(top4[:], topi[:])
            tt(rt4[:], r0_4[:], top4[:], op=Alu.subtract)
            tt(tph4[:], top4[:], sh4[:], op=Alu.add)

            iy4 = wp.tile([P, 4], f32, tag="iy4", name="iy4")
            ri12 = wp.tile([P, 12], f32, tag="ri12", name="ri12")
            rii = wp.tile([P, 12], i32, tag="rii", name="rii")
            ts(iy4[:], rt4[:], c_2p, None, op0=Alu.add)
            ts(iy4[:], iy4[:], 0.0, 511.0, op0=Alu.max, op1=Alu.min)
            ts(iy4[:], iy4[:], 512.0, None, op0=Alu.mult)
            tt(iy4[:], iy4[:], db4[:], op=Alu.add)
            for c in range(C):
                ts(_colview(ri12[:], c, 3, 4), iy4[:], float(c * HW), 0.0,
                   op0=Alu.add, op1=Alu.max)
            cpy(rii[:, 0:1], ri12[:, 0:1])
            cpy(rii[:, 1:12], ri12[:, 1:12])
            hpc.__exit__(None, None, None)

            # masks: sa = sign(iw-left+.5), sb = sign(lw-.5-iw); mxs = sa+sb
            # msk[k] = relu(mxs*vm[k] - vm[k]) = vm[k] * (left<=iw<lw)
            nlh4 = wp.tile([P, 4], f32, tag="nlh4", name="nlh4")
            lwh4 = wp.tile([P, 4], f32, tag="lwh4", name="lwh4")
            ts(nlh4[:], left4[:], -1.0, 0.5, op0=Alu.mult, op1=Alu.add)
            ts(lwh4[:], lw4[:], -0.5, None, op0=Alu.add)
            sa = wp.tile([P, S], f32, tag="sa", name="sa")
            sb = wp.tile([P, S], f32, tag="sb", name="sb")
            mxs = [wp.tile([P, S], f32, tag=f"mxs{i}", name=f"mxs{i}")
                   for i in range(BPC)]
            va = wp.tile([P, 2], f32, tag="va", name="va")
            vb = wp.tile([P, 2], f32, tag="vb", name="vb")
            vm = [wp.tile([P, 2], f32, tag=f"vm{i}", name=f"vm{i}")
                  for i in range(BPC)]
            nvm = [wp.tile([P, 2], f32, tag=f"nvm{i}", name=f"nvm{i}")
                   for i in range(BPC)]
            msk = [wp.tile([P, 2, S], f32, tag=f"msk{i}", name=f"msk{i}")
                   for i in range(BPC)]
            for i in range(BPC):
                act(sa[:], c_iw[:, :S], Act.Sign,
                    bias=nlh4[:, i : i + 1], scale=1.0)
                act(sb[:], c_iw[:, :S], Act.Sign,
                    bias=lwh4[:, i : i + 1], scale=-1.0)
                tt(mxs[i][:], sa[:], sb[:], op=Alu.add)
                ts(va[:], c_y2, top4[:, i : i + 1], None, op0=Alu.is_ge)
                ts(vb[:], c_y2, tph4[:, i : i + 1], None, op0=Alu.is_lt)
                tt(vm[i][:], va[:], vb[:], op=Alu.mult)
                ts(nvm[i][:], vm[i][:], -1.0, None, op0=Alu.mult)
                for k in range(2):
                    act(msk[i][:, k, :], mxs[i][:], Act.Relu,
                        scale=vm[i][:, k : k + 1],
                        bias=nvm[i][:, k : k + 1])
            # ---- gathers, mask-multiply, stores (per sample) --------------
            gat = [wp.tile([P, C, GL], f16, tag=f"gat{i}", name=f"gat{i}")
                   for i in range(BPC)]
            fin = [wp.tile([P, C, 2 * S], f32, tag=f"fin{i}", name=f"fin{i}")
                   for i in range(BPC)]
            for i in range(BPC):
                for c in range(C):
                    binst = nc.gpsimd.indirect_dma_start(
                        out=gat[i][:, c, :],
                        out_offset=None,
                        in_=bass.AP(x[:].tensor, 0,
                                    [[1, NEL - GL + 1], [1, GL]]),
                        in_offset=IndirectOffsetOnAxis(
                            ap=rii[:, 3 * i + c : 3 * i + c + 1], axis=0),
                    )
                    _patch_coef1(binst)
                for c in range(C):
                    g = gat[i][:, c, :]
                    gv = bass.AP(g.tensor, g.offset,
                                 [list(g.ap[0]), [W, 2], [1, S]])
                    fv = fin[i][:, c, :]
                    fvv = bass.AP(fv.tensor, fv.offset,
                                  [list(fv.ap[0]), [S, 2], [1, S]])
                    tt(fvv, gv, msk[i][:], op=Alu.mult)
                    ydst = bass.AP(y[:].tensor, (i * C + c) * SS,
                                   [[2 * S, P], [1, 2 * S]])
                    nc.sync.dma_start(ydst, fin[i][:, c, :])

            if debug:
                dbt = wp.tile([P, 80], f32, tag="dbt", name="dbt")
                nc.vector.memset(dbt[:], 0.0)
                cpy(dbt[:, 0:4], cand[:])
                cpy(dbt[:, 4:20], st1[:])
                cpy(dbt[:, 20:32], scw[:])
                cpy(dbt[:, 32:36], k1_4[:])
                cpy(dbt[:, 36:40], k2_4[:])
                cpy(dbt[:, 44:45], rf[:])
                cpy(dbt[:, 45:53], st2[:])
                cpy(dbt[:, 53:57], r0_4[:])
                cpy(dbt[:, 57:61], r1_4[:])
                cpy(dbt[:, 61:65], top4[:])
                cpy(dbt[:, 65:69], left4[:])
                cpy(dbt[:, 69:73], iy4[:])
                cpy(dbt[:, 73:77], db4[:])
                nc.sync.dma_start(dbg[:], dbt[:])
    nc.finalize()
    return nc


_CACHE: dict[str, object] = {}


def _const_maps() -> dict[str, np.ndarray]:
    packed = _consts()["c_all"]
    return {
        "c_allB": np.ascontiguousarray(packed[:, : _C_OFFS["c_iw"]]),
    }


def kernel(input1: np.ndarray, input2: np.ndarray, **_: np.ndarray) -> np.ndarray:
    input1 = np.ascontiguousarray(np.asarray(input1, dtype=np.float32))
    if "nc" not in _CACHE:
        _CACHE["nc"] = _build()
        _CACHE["consts"] = _const_maps()
    nc = _CACHE["nc"]
    consts = _CACHE["consts"]
    in_maps = [
        {"x": np.ascontiguousarray(input1[k * BPC : (k + 1) * BPC]), **consts}
        for k in range(N_CORES)
    ]
    res = run_bass_kernel_spmd(nc, in_maps, core_ids=list(range(N_CORES)))
    out = np.concatenate([r["y"] for r in res.results], axis=0)
    return out.astype(np.float32)


if __name__ == "__main__":
    rng = np.random.default_rng(1)
    xt = rng.standard_normal((B, CH_IN, H, W), dtype=np.float32)
    print(kernel(xt, np.zeros((B, C, S, S), np.float32)).shape)

# TIMING_PROBE
